# revision 7
# baseline (speedup 1.0000x reference)
"""Trainium2 Bass kernel for a Hamiltonian-NN symplectic gradient field.

Computes, for a 4-layer MLP H(x) = W3.(tanh(W2.(tanh(W1.(tanh(W0.x+b0))+b1))+b2))+b3,
the per-sample input gradient g = dH/dx and returns concat(g[:, 32:], -g[:, :32]).

Strategy: pure data-parallel over 8 NeuronCores (batch 65536 -> 8192/core).
On-chip, activations are kept *transposed* ([hidden, batch] with hidden on
partitions) so every hidden layer's matmul consumes the previous layer's
output directly as the moving operand, and the backward layers consume
host-pretransposed weight matrices as the stationary operand. The input is
shipped pre-transposed ([D, B_core]) and the output is produced transposed
([D, B_core]) with the symplectic rotation folded into a host-side
rearrangement of W0; the host does the cheap [*,64] transposes. This keeps
the device pipeline feed-forward (no PE transposes, no PE->DVE->PE chains).
Matmuls run in float32r (full PE rate at N=512, ~fp32 accuracy).
"""

import numpy as np

N_CORES = 8
D = 64          # input feature dim
H = 512         # hidden dim
BT = 512        # batch tile (one fp32 PSUM bank per [128, BT] tile)
K4 = H // 128   # 4 hidden sub-tiles

_CACHE = {}


def _build_program(b_core):
    import sys
    for p in ("/opt/trn_rl_repo",):
        if p not in sys.path:
            sys.path.append(p)
    from contextlib import ExitStack

    import concourse.tile as tile
    from concourse import bacc, mybir

    f32 = mybir.dt.float32
    f32r = mybir.dt.float32r
    Tanh = mybir.ActivationFunctionType.Tanh
    MULT = mybir.AluOpType.mult
    ADD = mybir.AluOpType.add

    nt = b_core // BT

    nc = bacc.Bacc(
        "TRN2",
        target_bir_lowering=False,
        debug=False,
        enable_asserts=True,
        num_devices=N_CORES,
    )

    # DRAM I/O (x and out are transposed: [D, b_core])
    xt_d = nc.dram_tensor("xt", [D, b_core], f32r, kind="ExternalInput").ap()
    w0_d = nc.dram_tensor("w0", [D, H], f32r, kind="ExternalInput").ap()
    w1_d = nc.dram_tensor("w1", [H, H], f32r, kind="ExternalInput").ap()
    w2_d = nc.dram_tensor("w2", [H, H], f32r, kind="ExternalInput").ap()
    w1t_d = nc.dram_tensor("w1t", [H, H], f32r, kind="ExternalInput").ap()
    w2t_d = nc.dram_tensor("w2t", [H, H], f32r, kind="ExternalInput").ap()
    w0r_d = nc.dram_tensor("w0r", [H, D], f32r, kind="ExternalInput").ap()
    b0_d = nc.dram_tensor("b0c", [128, K4], f32, kind="ExternalInput").ap()
    b1_d = nc.dram_tensor("b1c", [128, K4], f32, kind="ExternalInput").ap()
    b2_d = nc.dram_tensor("b2c", [128, K4], f32, kind="ExternalInput").ap()
    w3_d = nc.dram_tensor("w3c", [128, K4], f32, kind="ExternalInput").ap()
    w3n_d = nc.dram_tensor("w3nc", [128, K4], f32, kind="ExternalInput").ap()
    out_d = nc.dram_tensor("out", [D, b_core], f32, kind="ExternalOutput").ap()

    with tile.TileContext(nc) as tc:
        with ExitStack() as ctx:
            const = ctx.enter_context(tc.tile_pool(name="const", bufs=1))

            # forward weights first: the first tile's compute needs only these
            w0 = const.tile([D, H], f32r, tag="w0")
            nc.sync.dma_start(w0[:], w0_d[:])
            b0c = const.tile([128, K4], f32, tag="b0c")
            b1c = const.tile([128, K4], f32, tag="b1c")
            b2c = const.tile([128, K4], f32, tag="b2c")
            w3c = const.tile([128, K4], f32, tag="w3c")
            w3nc = const.tile([128, K4], f32, tag="w3nc")
            nc.sync.dma_start(b0c[:], b0_d[:])
            nc.sync.dma_start(b1c[:], b1_d[:])
            nc.sync.dma_start(b2c[:], b2_d[:])
            nc.sync.dma_start(w3c[:], w3_d[:])
            nc.sync.dma_start(w3nc[:], w3n_d[:])
            w1 = [const.tile([128, H], f32r, tag=f"w1_{k}", name=f"w1_{k}") for k in range(K4)]
            w2 = [const.tile([128, H], f32r, tag=f"w2_{k}", name=f"w2_{k}") for k in range(K4)]
            w1t = [const.tile([128, H], f32r, tag=f"w1t_{k}", name=f"w1t_{k}") for k in range(K4)]
            w2t = [const.tile([128, H], f32r, tag=f"w2t_{k}", name=f"w2t_{k}") for k in range(K4)]
            w0r = [const.tile([128, D], f32r, tag=f"w0r_{k}", name=f"w0r_{k}") for k in range(K4)]
            for k in range(K4):
                sl = slice(k * 128, (k + 1) * 128)
                nc.sync.dma_start(w1[k][:], w1_d[sl, :])
                nc.sync.dma_start(w2[k][:], w2_d[sl, :])
            for k in range(K4):
                sl = slice(k * 128, (k + 1) * 128)
                nc.sync.dma_start(w1t[k][:], w1t_d[sl, :])
                nc.sync.dma_start(w2t[k][:], w2t_d[sl, :])
                nc.sync.dma_start(w0r[k][:], w0r_d[sl, :])

            xt_p = ctx.enter_context(tc.tile_pool(name="xt", bufs=3))
            oT_p = ctx.enter_context(tc.tile_pool(name="oT", bufs=3))
            h0_p = ctx.enter_context(tc.tile_pool(name="h0", bufs=6))
            h1_p = ctx.enter_context(tc.tile_pool(name="h1", bufs=6))
            h2_p = ctx.enter_context(tc.tile_pool(name="h2", bufs=3))
            s_p = ctx.enter_context(tc.tile_pool(name="s", bufs=4))
            c0_p = ctx.enter_context(tc.tile_pool(name="c0", bufs=6))
            c1_p = ctx.enter_context(tc.tile_pool(name="c1", bufs=6))
            g2_p = ctx.enter_context(tc.tile_pool(name="g2", bufs=5))
            g1_p = ctx.enter_context(tc.tile_pool(name="g1", bufs=5))
            g0_p = ctx.enter_context(tc.tile_pool(name="g0", bufs=6))

            ps_acc = ctx.enter_context(tc.tile_pool(name="ps_acc", bufs=6, space="PSUM"))
            ps_oT = ctx.enter_context(tc.tile_pool(name="ps_oT", bufs=2, space="PSUM"))

            for bt in range(nt):
                bsl = slice(bt * BT, (bt + 1) * BT)

                # ---- load input slice (already transposed on host) ----
                xt = xt_p.tile([D, BT], f32r, tag="xt", name=f"xt_{bt}")
                nc.sync.dma_start(xt[:], xt_d[:, bsl])

                # ---- forward layer 0 ----
                h0 = []
                for m in range(K4):
                    z = ps_acc.tile([128, BT], f32, tag="acc", name=f"z0_{bt}_{m}")
                    nc.tensor.matmul(
                        z[:], w0[:, m * 128:(m + 1) * 128], xt[:],
                        start=True, stop=True,
                    )
                    h = h0_p.tile([128, BT], f32r, tag="h0", name=f"h0_{bt}_{m}")
                    nc.scalar.activation(h[:], z[:], Tanh, bias=b0c[:, m:m + 1], scale=1.0)
                    h0.append(h)

                # ---- forward layer 1 (m-outer, k-inner) ----
                h1 = []
                for m in range(K4):
                    z = ps_acc.tile([128, BT], f32, tag="acc", name=f"z1_{bt}_{m}")
                    for k in range(K4):
                        nc.tensor.matmul(
                            z[:], w1[k][:, m * 128:(m + 1) * 128], h0[k][:],
                            start=(k == 0), stop=(k == K4 - 1),
                        )
                    h = h1_p.tile([128, BT], f32r, tag="h1", name=f"h1_{bt}_{m}")
                    nc.scalar.activation(h[:], z[:], Tanh, bias=b1c[:, m:m + 1], scale=1.0)
                    h1.append(h)

                # ---- forward layer 2 + initial backward grad ----
                g2 = []
                for m in range(K4):
                    z = ps_acc.tile([128, BT], f32, tag="acc", name=f"z2_{bt}_{m}")
                    for k in range(K4):
                        nc.tensor.matmul(
                            z[:], w2[k][:, m * 128:(m + 1) * 128], h1[k][:],
                            start=(k == 0), stop=(k == K4 - 1),
                        )
                    h = h2_p.tile([128, BT], f32, tag="h2", name=f"h2_{bt}_{m}")
                    nc.scalar.activation(h[:], z[:], Tanh, bias=b2c[:, m:m + 1], scale=1.0)
                    s = s_p.tile([128, BT], f32, tag="s", name=f"s2_{bt}_{m}")
                    nc.vector.tensor_tensor(s[:], h[:], h[:], MULT)
                    # g2 = W3 * (1 - h2^2) = (h2^2) * (-W3) + W3
                    g = g2_p.tile([128, BT], f32r, tag="g2", name=f"g2_{bt}_{m}")
                    nc.vector.tensor_scalar(
                        g[:], s[:], w3nc[:, m:m + 1], w3c[:, m:m + 1], MULT, ADD
                    )
                    g2.append(g)

                # deferred tanh' factors: ACT squares run while PE does the
                # backward matmuls; DVE affines feed the backward multiplies.
                c1 = []
                for m in range(K4):
                    s = s_p.tile([128, BT], f32, tag="s", name=f"s1_{bt}_{m}")
                    nc.scalar.square(s[:], h1[m][:])
                    c = c1_p.tile([128, BT], f32, tag="c1", name=f"c1_{bt}_{m}")
                    nc.vector.tensor_scalar(c[:], s[:], -1.0, 1.0, MULT, ADD)
                    c1.append(c)

                # ---- backward layer 2: d1 = W2 @ g2; g1 = d1 * c1 ----
                g1 = []
                for m in range(K4):
                    dps = ps_acc.tile([128, BT], f32, tag="acc", name=f"d1_{bt}_{m}")
                    for k in range(K4):
                        nc.tensor.matmul(
                            dps[:], w2t[k][:, m * 128:(m + 1) * 128], g2[k][:],
                            start=(k == 0), stop=(k == K4 - 1),
                        )
                    g = g1_p.tile([128, BT], f32r, tag="g1", name=f"g1_{bt}_{m}")
                    nc.vector.tensor_tensor(g[:], dps[:], c1[m][:], MULT)
                    g1.append(g)

                c0 = []
                for m in range(K4):
                    s = s_p.tile([128, BT], f32, tag="s", name=f"s0_{bt}_{m}")
                    nc.scalar.square(s[:], h0[m][:])
                    c = c0_p.tile([128, BT], f32, tag="c0", name=f"c0_{bt}_{m}")
                    nc.vector.tensor_scalar(c[:], s[:], -1.0, 1.0, MULT, ADD)
                    c0.append(c)

                # ---- backward layer 1: d0 = W1 @ g1; g0 = d0 * c0 ----
                g0 = []
                for m in range(K4):
                    dps = ps_acc.tile([128, BT], f32, tag="acc", name=f"d0_{bt}_{m}")
                    for k in range(K4):
                        nc.tensor.matmul(
                            dps[:], w1t[k][:, m * 128:(m + 1) * 128], g1[k][:],
                            start=(k == 0), stop=(k == K4 - 1),
                        )
                    g = g0_p.tile([128, BT], f32r, tag="g0", name=f"g0_{bt}_{m}")
                    nc.vector.tensor_tensor(g[:], dps[:], c0[m][:], MULT)
                    g0.append(g)

                # ---- backward layer 0 (transposed out, symplectic folded) ----
                # outT[f, b] = sum_k w0r[k, f] * g0[k, b]
                oT_ps = ps_oT.tile([D, BT], f32, tag="oT_ps", name=f"oTp_{bt}")
                for k in range(K4):
                    nc.tensor.matmul(
                        oT_ps[:], w0r[k][:], g0[k][:],
                        start=(k == 0), stop=(k == K4 - 1),
                    )
                oT = oT_p.tile([D, BT], f32, tag="oT", name=f"oT_{bt}")
                nc.vector.tensor_copy(oT[:], oT_ps[:])
                nc.sync.dma_start(out_d[:, bsl], oT[:])

    nc.compile()
    return nc


def _host_weights(inputs):
    w0 = np.ascontiguousarray(np.asarray(inputs["W0"], np.float32))
    w1 = np.ascontiguousarray(np.asarray(inputs["W1"], np.float32))
    w2 = np.ascontiguousarray(np.asarray(inputs["W2"], np.float32))
    w3 = np.asarray(inputs["W3"], np.float32).reshape(H)
    b0 = np.asarray(inputs["b0"], np.float32)
    b1 = np.asarray(inputs["b1"], np.float32)
    b2 = np.asarray(inputs["b2"], np.float32)

    w0m = np.ascontiguousarray(w0.T)  # [H, D]
    n = D // 2
    w0r = np.concatenate([w0m[:, n:], -w0m[:, :n]], axis=1)  # symplectic fold

    def cols(v):
        return np.ascontiguousarray(v.reshape(K4, 128).T)

    return {
        "w0": w0,
        "w1": w1,
        "w2": w2,
        "w1t": np.ascontiguousarray(w1.T),
        "w2t": np.ascontiguousarray(w2.T),
        "w0r": np.ascontiguousarray(w0r),
        "b0c": cols(b0),
        "b1c": cols(b1),
        "b2c": cols(b2),
        "w3c": cols(w3),
        "w3nc": cols(-w3),
    }


def _in_maps(inputs):
    x = np.asarray(inputs["x"], np.float32)
    b_core = x.shape[0] // N_CORES
    w = _host_weights(inputs)
    maps = []
    for i in range(N_CORES):
        m = {"xt": np.ascontiguousarray(x[i * b_core:(i + 1) * b_core].T)}
        m.update(w)
        maps.append(m)
    return maps, b_core


def kernel(**inputs):
    import sys
    for p in ("/opt/trn_rl_repo",):
        if p not in sys.path:
            sys.path.append(p)
    from concourse.bass_utils import run_bass_kernel_spmd

    in_maps, b_core = _in_maps(inputs)
    if b_core not in _CACHE:
        _CACHE[b_core] = _build_program(b_core)
    nc = _CACHE[b_core]

    res = run_bass_kernel_spmd(nc, in_maps, core_ids=list(range(N_CORES)))
    out = np.empty((b_core * N_CORES, D), np.float32)
    for i in range(N_CORES):
        out[i * b_core:(i + 1) * b_core] = res.results[i]["out"].T
    return out
